# revision 29
# baseline (speedup 1.0000x reference)
"""Multi-head attention Bass/Tile kernel for TRN2, 8-core SPMD.

Sharding: core c handles batch b = c//2, query-half qh = c%2. The host
rotates the token axis per core so query rows sit at [0:TQ] (attention is
key-permutation invariant), and gathers the unmasked keys (mask compaction)
so K/V projection + attention only touch TK <= T key tokens.

The kernel is organized into 16 "ticks" of (ic, hp) = (query i-chunk,
head pair), paced by the softmax exp stream on ACT (~145us). Each tick
emits its S matmuls + exps ONE KEY-TILE AT A TIME with jobs woven in
between: the PV/den matmuls of the previous tick, projection chunks and
out-proj/LayerNorm tiles. The fine interleave matters because the PE
queue is strict FIFO and s-tiles (psA) drain at ACT's exp pace — a
batch of S matmuls would park the PE on a psA slot with runnable work
stuck behind it. Projections draw PSUM from a separate pool (psP) for
the same reason. PSUM budget: psA 2x2 banks + psP 2 + psB 2 = 8.

Projections (Q/K/V/O) run as fp8e4m3 MatmulPerfMode.DoubleRow (two
128-contraction subtiles per matmul at 0.5 cyc/row; full 128x128 tile —
DR is incompatible with col tiling). Weights are pre-scaled by 32 on
the host so U(-1/32,1/32) lands in fp8's normal range; inverse scales
fold into the PSUM->SBUF copies. Scores/exp/PV/den stay bf16.

LayerNorm statistics ride the DVE: the residual add and an x^2 pass
carry accum_out (per-partition free-dim sums), so ACT only does the
tiny [128,1] Ln/Exp for rstd. When ln_g==1 and ln_b==0 (detected on
host), the final affine is a single tensor_scalar.

Matmul layouts (out = lhsT.T @ rhs, contraction on partitions):
  QT/KT [F, *] bf16 : lhsT=w*T [128,2,128] fp8 DR, rhs=hT* [128,2,*] fp8
  V     [TK, F] bf16: lhsT=hTk [128,2,t128] fp8 DR, rhs=wvT [128,2,F]
  S^T   [j, (h0 i512 | h1 i512)] psum (2 banks): row-tiled head pair
  exp   one ACT op per j-tile: [128, 1024], bias=maskbias per-partition
  PV+den [d0:64|den 64:128, i] psum: lhsT=V[j,64]@(0,0) + ones[j,64]@(0,64)
  O     [t, D] psum : lhsT=AVT [128,2,t128] fp8 DR, rhs=woT [128,2,D]
"""
import numpy as np
import ml_dtypes

import concourse.bass as bass
import concourse.tile as tile
from concourse import bacc, mybir

F32 = mybir.dt.float32
BF16 = mybir.dt.bfloat16
FP8 = mybir.dt.float8e4
AF = mybir.ActivationFunctionType
ALU = mybir.AluOpType
DR = mybir.MatmulPerfMode.DoubleRow

NEG_BIG = -1.0e30
WS = 32.0          # host-side fp8 weight scale (wq/wk/wv/wo * 32)
AS = 8.0           # device-side AVT fp8 scale


def _pin_act_tables():
    """Force every ACT func we use (Exp, Ln, Copy) to resolve to the
    single `natural_log_exp_and_others` table set, so the kernel does
    exactly one ACT_TABLE_LOAD instead of thrashing (~2.6us per switch).
    Preserves dict order (set ids are positional)."""
    import concourse.hw_specs as hw_specs
    if getattr(hw_specs, "_mha_tables_pinned", False):
        return
    orig = hw_specs.get_activation_tables

    def patched(module_arch):
        tabs = orig(module_arch)
        pin = "natural_log_exp_and_others"
        if pin in tabs:
            pinned_funcs = tabs[pin]
            for name, fns in tabs.items():
                if name != pin:
                    tabs[name] = fns - pinned_funcs
        return tabs

    hw_specs.get_activation_tables = patched
    import concourse.bacc as bacc_mod
    bacc_mod.get_activation_tables = patched
    hw_specs._mha_tables_pinned = True


def _chunks(total, step):
    out = []
    off = 0
    while off < total:
        out.append((off, min(step, total - off)))
        off += step
    return out


def build_nc(T, TQ, TK, D, NH, DH, n_cores=8, plain_ln=True, debug=False):
    """Build the single-core SPMD Bass program. TK = compacted key count.
    plain_ln: ln_g is all-ones and ln_b all-zeros (skip the affine)."""
    F = NH * DH
    DC = D // 128        # D contraction chunks
    DP = DC // 2         # DoubleRow dc pairs
    FC = F // 128        # feature chunks (2 heads per chunk, DH=64)
    FP = FC // 2         # DoubleRow fc pairs
    KC = TK // 128       # key tiles
    TT = TQ // 128       # query t-tiles
    ICS = min(512, TQ)   # i-chunk size
    ICN = TQ // ICS
    FS = min(512, F)
    DS = min(512, D)
    assert DH == 64 and F % 128 == 0 and D % 128 == 0
    assert TQ % 128 == 0 and TK % 128 == 0 and DC % 2 == 0 and FC % 2 == 0

    _pin_act_tables()
    nc = bacc.Bacc("TRN2", target_bir_lowering=False, debug=debug,
                   num_devices=n_cores)

    # ---- DRAM I/O ----
    hTq_d = nc.dram_tensor("hTq", [128, DC, TQ], FP8, kind="ExternalInput")
    hTk_d = nc.dram_tensor("hTk", [128, DC, TK], FP8, kind="ExternalInput")
    hq_d = nc.dram_tensor("hq", [TQ, D], F32, kind="ExternalInput")
    wqT_d = nc.dram_tensor("wqT", [128, DC, F], FP8, kind="ExternalInput")
    wkT_d = nc.dram_tensor("wkT", [128, DC, F], FP8, kind="ExternalInput")
    wvT_d = nc.dram_tensor("wvT", [128, DC, F], FP8, kind="ExternalInput")
    woT_d = nc.dram_tensor("woT", [128, FC, D], FP8, kind="ExternalInput")
    mb_d = nc.dram_tensor("maskbias", [128, KC], F32, kind="ExternalInput")
    g_d = nc.dram_tensor("g_rep", [128, D], F32, kind="ExternalInput")
    b_d = nc.dram_tensor("b_rep", [128, D], F32, kind="ExternalInput")
    out_d = nc.dram_tensor("out", [TQ, D], BF16, kind="ExternalOutput")

    with tile.TileContext(nc) as tc:
        with (
            tc.tile_pool(name="hpool", bufs=1) as hpool,
            tc.tile_pool(name="wts", bufs=4) as wts,
            tc.tile_pool(name="acts", bufs=1) as acts,
            tc.tile_pool(name="small", bufs=1) as small,
            tc.tile_pool(name="exps", bufs=2) as expp,
            tc.tile_pool(name="epi", bufs=5) as epi,
            tc.tile_pool(name="psA", bufs=2, space="PSUM") as psA,
            tc.tile_pool(name="psP", bufs=2, space="PSUM") as psP,
            tc.tile_pool(name="psB", bufs=2, space="PSUM") as psB,
        ):
            # ---- persistent SBUF tiles ----
            hTq = hpool.tile([128, DC, TQ], FP8, tag="htq")
            hTk = hpool.tile([128, DC, TK], FP8, tag="htk")
            wqT = wts.tile([128, DC, F], FP8, tag="w")
            wkT = wts.tile([128, DC, F], FP8, tag="w")
            wvT = wts.tile([128, DC, F], FP8, tag="w")
            woT = wts.tile([128, FC, D], FP8, tag="w")
            QT = acts.tile([128, FC * TQ], BF16, tag="qt")
            KT = acts.tile([128, FC * TK], BF16, tag="kt")
            V = acts.tile([128, KC, F], BF16, tag="v")
            AVT = acts.tile([128, FC, TQ], FP8, tag="avt")
            ones = small.tile([128, 64], BF16, tag="ones")
            mb = small.tile([128, KC], F32, tag="mb")
            eps_t = small.tile([128, 1], F32, tag="eps")

            nc.vector.memset(ones[:], 1.0)
            nc.vector.memset(eps_t[:], 1e-5)
            nc.sync.dma_start(mb[:], mb_d[:])
            # The DRAM tensors are pre-arranged on the host in SBUF layout
            # ([128, DC, *]) so each loads in few DMAs; queues are spread
            # so Q inputs (sync/scalar) and K inputs (gpsimd) run on
            # parallel channels. The p=0 slices of wq/wk (128 cols) go
            # first so the p0 projections — and with them the ACT exp
            # stream — start as soon as hTq/hTk land, instead of waiting
            # for the full weight tensors.
            nc.sync.dma_start(wqT[:, :, 0:128], wqT_d[:, :, 0:128])
            nc.sync.dma_start(wkT[:, :, 0:128], wkT_d[:, :, 0:128])
            nc.scalar.dma_start(hTq[:], hTq_d[:])
            nc.sync.dma_start(hTk[:], hTk_d[:])
            nc.sync.dma_start(wqT[:, :, 128:F], wqT_d[:, :, 128:F])
            nc.sync.dma_start(wkT[:, :, 128:F], wkT_d[:, :, 128:F])
            nc.scalar.dma_start(wvT[:], wvT_d[:])
            # sync-queue DMAs run serially, so woT at the tail is
            # naturally deprioritized out of the critical startup burst
            # (it isn't needed until the first oln, ~halfway in).
            nc.sync.dma_start(woT[:], woT_d[:])
            g_re = small.tile([128, D], F32, tag="g")
            b_re = small.tile([128, D], F32, tag="b")
            if not plain_ln:
                nc.sync.dma_start(g_re[:], g_d[:])
                nc.sync.dma_start(b_re[:], b_d[:])

            # ---- emission helpers (fp8 DoubleRow projections) ----
            # Projections and the out-projection draw PSUM from psP so
            # they never contend with the s-tiles in psA: the PE queue is
            # strict FIFO, so an S matmul parked on an exp-paced psA slot
            # would otherwise stall every instruction emitted behind it.
            def qchunk(p, t0, tn):
                def job():
                    ps = psP.tile([128, tn], F32, tag="P")
                    for dp in range(DP):
                        nc.tensor.matmul(
                            ps[:],
                            wqT[:, 2 * dp:2 * dp + 2, p * 128:(p + 1) * 128],
                            hTq[:, 2 * dp:2 * dp + 2, t0:t0 + tn],
                            start=(dp == 0), stop=(dp == DP - 1),
                            perf_mode=DR)
                    # fold 1/WS (fp8 weight scale) and 1/8 (attn scale)
                    nc.vector.tensor_scalar(
                        QT[:, p * TQ + t0: p * TQ + t0 + tn], ps[:],
                        1.0 / (WS * 8.0), None, op0=ALU.mult)
                return job

            def kchunk(p, t0, tn):
                def job():
                    ps = psP.tile([128, tn], F32, tag="P")
                    for dp in range(DP):
                        nc.tensor.matmul(
                            ps[:],
                            wkT[:, 2 * dp:2 * dp + 2, p * 128:(p + 1) * 128],
                            hTk[:, 2 * dp:2 * dp + 2, t0:t0 + tn],
                            start=(dp == 0), stop=(dp == DP - 1),
                            perf_mode=DR)
                    nc.vector.tensor_scalar(
                        KT[:, p * TK + t0: p * TK + t0 + tn], ps[:],
                        1.0 / WS, None, op0=ALU.mult)
                return job

            def vchunk(jc, f0, fn):
                def job():
                    ps = psP.tile([128, fn], F32, tag="P")
                    for dp in range(DP):
                        nc.tensor.matmul(
                            ps[:],
                            hTk[:, 2 * dp:2 * dp + 2, jc * 128:(jc + 1) * 128],
                            wvT[:, 2 * dp:2 * dp + 2, f0:f0 + fn],
                            start=(dp == 0), stop=(dp == DP - 1),
                            perf_mode=DR)
                    nc.vector.tensor_scalar(
                        V[:, jc, f0:f0 + fn], ps[:],
                        1.0 / WS, None, op0=ALU.mult)
                return job

            def emit_s(ic, hp, e3, jc):
                """One key-tile of scores + its exp."""
                io = ic * ICS
                s = psA.tile([128, 2 * ICS], F32, tag="A")
                nc.tensor.matmul(
                    s[:, 0:ICS],
                    KT[0:64, hp * TK + jc * 128: hp * TK + (jc + 1) * 128],
                    QT[0:64, hp * TQ + io: hp * TQ + io + ICS],
                    start=True, stop=True, tile_position=(0, 0))
                nc.tensor.matmul(
                    s[:, ICS:2 * ICS],
                    KT[64:128, hp * TK + jc * 128: hp * TK + (jc + 1) * 128],
                    QT[64:128, hp * TQ + io: hp * TQ + io + ICS],
                    start=True, stop=True, tile_position=(64, 0))
                nc.scalar.activation(e3[:, jc, :], s[:], AF.Exp,
                                     bias=mb[:, jc:jc + 1])

            def pv_jobs(ic, hp, e3):
                """Per-key-tile PV + denominator jobs, then the AVT
                normalize, for a tick whose exps are (nearly) done."""
                io = ic * ICS
                h0, h1 = 2 * hp, 2 * hp + 1
                pvP = psB.tile([128, ICS], F32, tag="pv")
                pvD = psB.tile([128, ICS], F32, tag="pv")
                jobs = []
                KP = KC // 2
                for jc in range(KC):
                    def j(jc=jc):
                        st, sp = (jc == 0), (jc == KC - 1)
                        nc.tensor.matmul(
                            pvP[0:64, :],
                            V[:, jc, h0 * DH:(h0 + 1) * DH],
                            e3[:, jc, 0:ICS], start=st, stop=sp,
                            tile_position=(0, 0), skip_group_check=True)
                        nc.tensor.matmul(
                            pvP[64:128, :],
                            V[:, jc, h1 * DH:(h1 + 1) * DH],
                            e3[:, jc, ICS:2 * ICS], start=st, stop=sp,
                            tile_position=(0, 64), skip_group_check=True)
                    jobs.append(j)

                # Pair-sum adjacent e-tiles IN PLACE on the DVE (idle
                # capacity; the PV matmuls above already consumed the
                # original slices) so the PE-side denominator reduction
                # needs 5 matmul rounds instead of 9.
                def mk_pair(k):
                    def j():
                        nc.vector.tensor_tensor(
                            e3[:, 2 * k, :], e3[:, 2 * k, :],
                            e3[:, 2 * k + 1, :], op=ALU.add)
                    return j
                jobs += [mk_pair(k) for k in range(KP)]

                def denjob():
                    idxs = [2 * k for k in range(KP)]
                    if 2 * KP != KC:
                        idxs.append(KC - 1)
                    for k, idx in enumerate(idxs):
                        st, sp = (k == 0), (k == len(idxs) - 1)
                        nc.tensor.matmul(
                            pvD[0:64, :], ones[:, 0:64],
                            e3[:, idx, 0:ICS], start=st, stop=sp,
                            tile_position=(0, 0), skip_group_check=True)
                        nc.tensor.matmul(
                            pvD[64:128, :], ones[:, 0:64],
                            e3[:, idx, ICS:2 * ICS], start=st, stop=sp,
                            tile_position=(0, 64), skip_group_check=True)
                jobs.append(denjob)

                def norm():
                    # approx-reciprocal of den then one multiply for both
                    # heads; AVT is fp8 scaled by AS (recip gets AS/den),
                    # reading PV straight from PSUM.
                    pvDc = epi.tile([128, ICS], F32, tag="rec")
                    nc.vector.tensor_scalar(pvDc[:], pvD[:], 1.0 / AS, None,
                                            op0=ALU.mult)
                    nc.vector.reciprocal_approx_fast(pvDc[:], pvDc[:])
                    nc.vector.tensor_tensor(
                        AVT[:, hp, io:io + ICS],
                        pvP[:], pvDc[:], op=ALU.mult)
                jobs.append(norm)
                return jobs

            def oln_jobs(tt, drain=False):
                """Out-projection + residual + LayerNorm for one t-tile,
                as two jobs (one per half-D chunk). LN stats come from
                DVE accum_out; ACT only does the tiny Ln/Exp for rstd."""
                st = {}
                dchunks = _chunks(D, DS)

                def ochunk(ci, d0, dn):
                    ps = psP.tile([128, dn], F32, tag="P")
                    for fp_ in range(FP):
                        nc.tensor.matmul(
                            ps[:],
                            AVT[:, 2 * fp_:2 * fp_ + 2,
                                tt * 128:(tt + 1) * 128],
                            woT[:, 2 * fp_:2 * fp_ + 2, d0:d0 + dn],
                            start=(fp_ == 0), stop=(fp_ == FP - 1),
                            perf_mode=DR)
                    # x = attn_out + h; 1/(WS*AS) undoes wo and AVT scales
                    nc.vector.scalar_tensor_tensor(
                        st['x'][:, d0:d0 + dn], ps[:], 1.0 / (WS * AS),
                        st['hqt'][:, d0:d0 + dn], op0=ALU.mult, op1=ALU.add,
                        accum_out=st['stats'][:, ci:ci + 1])

                def a():
                    st['x'] = epi.tile([128, D], F32, tag="x", name="x")
                    st['hqt'] = epi.tile([128, D], F32, tag="hqt", name="hqt")
                    st['stats'] = epi.tile([128, 8], F32, tag="stats", name="stats")
                    nc.sync.dma_start(st['hqt'][:],
                                      hq_d[tt * 128:(tt + 1) * 128, :])
                    ochunk(0, *dchunks[0])

                def b():
                    x, hqt, stats = st['x'], st['hqt'], st['stats']
                    for ci, (d0, dn) in enumerate(dchunks[1:], start=1):
                        ochunk(ci, d0, dn)
                    # sum(x^2) (uncentered variance); hqt is dead, use it
                    # as the mandatory elementwise output scratch. In the
                    # drain, ACT is idle (exp stream over) while the DVE
                    # LN chains serialize — run the x^2 pass there.
                    if drain:
                        nc.scalar.activation(hqt[:], x[:], AF.Square,
                                             accum_out=stats[:, 2:3])
                    else:
                        nc.vector.scalar_tensor_tensor(
                            hqt[:], x[:], 1.0, x[:],
                            op0=ALU.mult, op1=ALU.mult,
                            accum_out=stats[:, 2:3])
                    mu = stats[:, 3:4]
                    nc.vector.tensor_tensor(mu, stats[:, 0:1],
                                            stats[:, 1:2], op=ALU.add)
                    nc.vector.tensor_scalar(mu, mu, 1.0 / D, None,
                                            op0=ALU.mult)
                    mu2 = stats[:, 4:5]
                    nc.vector.tensor_scalar(mu2, mu, mu, None, op0=ALU.mult)
                    msq = stats[:, 5:6]
                    nc.vector.tensor_scalar(msq, stats[:, 2:3], 1.0 / D,
                                            None, op0=ALU.mult)
                    var = stats[:, 6:7]
                    nc.vector.tensor_tensor(var, msq, mu2, op=ALU.subtract)
                    # rstd = exp(-0.5*ln(var+eps)): Ln+Exp live in one ACT
                    # table set with the attention Exps -> no table thrash
                    lnv = stats[:, 0:1]
                    nc.scalar.activation(lnv, var, AF.Ln, bias=eps_t[:])
                    rstd = stats[:, 1:2]
                    nc.scalar.activation(rstd, lnv, AF.Exp, scale=-0.5)
                    nmr = stats[:, 4:5]
                    nc.vector.tensor_scalar(nmr, mu, rstd, -1.0,
                                            op0=ALU.mult, op1=ALU.mult)
                    # xhat = x*rstd - mu*rstd into a bf16 staging tile
                    # (halves the output DMA bytes), in half-D pieces so
                    # the first out-DMA overlaps the second affine. Drain
                    # tiles compute xhat on the now-idle ACT engine
                    # (Identity with per-partition scale/bias) to shorten
                    # the serialized DVE tail.
                    xo = epi.tile([128, D], BF16, tag="xo", name="xo")
                    for ci, (d0, dn) in enumerate(dchunks):
                        if plain_ln and drain and ci == 0:
                            nc.scalar.activation(
                                xo[:, d0:d0 + dn], x[:, d0:d0 + dn],
                                AF.Identity, scale=rstd, bias=nmr)
                        elif plain_ln:
                            nc.vector.tensor_scalar(
                                xo[:, d0:d0 + dn], x[:, d0:d0 + dn], rstd,
                                nmr, op0=ALU.mult, op1=ALU.add)
                        else:
                            nc.vector.tensor_scalar(
                                x[:, d0:d0 + dn], x[:, d0:d0 + dn], rstd,
                                nmr, op0=ALU.mult, op1=ALU.add)
                            nc.vector.scalar_tensor_tensor(
                                x[:, d0:d0 + dn], x[:, d0:d0 + dn], 1.0,
                                g_re[:, d0:d0 + dn],
                                op0=ALU.mult, op1=ALU.mult)
                            nc.vector.tensor_tensor(
                                xo[:, d0:d0 + dn], x[:, d0:d0 + dn],
                                b_re[:, d0:d0 + dn], op=ALU.add)
                        nc.sync.dma_start(
                            out_d[tt * 128:(tt + 1) * 128, d0:d0 + dn],
                            xo[:, d0:d0 + dn])
                return [a, b]

            # ---- ACT-anchored tick schedule with fine interleave ----
            # warmup: QT/KT chunk 0 so tick (0,0)'s scores start early
            for t0, tn in _chunks(TQ, 512):
                qchunk(0, t0, tn)()
            for t0, tn in _chunks(TK, 512):
                kchunk(0, t0, tn)()

            ticks = [(ic, hp) for ic in range(ICN) for hp in range(FC)]
            # V-proj chunks: f-half 0 is needed by PV of hp 0..FC/2-1,
            # f-half 1 by hp FC/2.. — front-load half 0, spread half 1.
            vjobs1 = [(jc, FS, F - FS) for jc in range(KC)] if F > FS else []
            prev = None
            for t, (ic, hp) in enumerate(ticks):
                jobs = []
                if prev is not None:
                    jobs += pv_jobs(*prev)
                if ic == 0:
                    if hp == 0:
                        jobs += [vchunk(jc, 0, FS) for jc in range(KC)]
                    elif hp in (1, 2, 3) and vjobs1:
                        k = (len(vjobs1) + 2) // 3
                        jobs += [vchunk(*vj)
                                 for vj in vjobs1[(hp - 1) * k: hp * k]]
                    if hp + 1 < FC:
                        jobs += [qchunk(hp + 1, t0, tn)
                                 for t0, tn in _chunks(TQ, 512)]
                        jobs += [kchunk(hp + 1, t0, tn)
                                 for t0, tn in _chunks(TK, 512)]
                if ic >= 1 and hp % 2 == 1:
                    # olns of the previous ic: 4 t-tiles over ticks 1,3,5,7
                    jobs += oln_jobs((ic - 1) * ICS // 128 + (hp - 1) // 2)
                # Emit S one key-tile at a time with jobs woven between,
                # so the exp-paced psA slots never park the PE FIFO while
                # runnable work sits behind them.
                e3 = expp.tile([128, KC, 2 * ICS], BF16, tag="e")
                emit_s(ic, hp, e3, 0)
                if KC > 1:
                    emit_s(ic, hp, e3, 1)
                ngaps = max(KC - 2, 1)
                per = -(-len(jobs) // ngaps) if jobs else 0
                gi = 0
                for jc in range(2, KC):
                    for job in jobs[gi:gi + per]:
                        job()
                    gi += per
                    emit_s(ic, hp, e3, jc)
                for job in jobs[gi:]:
                    job()
                prev = (ic, hp, e3)
            # drain: PV of the last tick, then the final ic's olns
            for job in pv_jobs(*prev):
                job()
            drain_jobs = [oln_jobs(tt, drain=True)
                          for tt in range((ICN - 1) * ICS // 128,
                                          TQ // 128)]
            for a, _ in drain_jobs:
                a()
            for _, b in drain_jobs:
                b()

    nc.compile()
    return nc


def choose_tk(attn_mask):
    """Compacted key count: max unmasked count over batches, ceil to 128."""
    m = np.asarray(attn_mask)
    counts = (~m).sum(axis=0)
    tk = int(((int(counts.max()) + 127) // 128) * 128)
    return max(tk, 128)


def host_prep_core(c, tk, h, attn_mask, wq, wkv, wo, ln_g, ln_b, NH=16, DH=64):
    """Build the per-core input map (numpy) for core c."""
    T, B, D = h.shape
    F = NH * DH
    TQ = T // 2
    KC = tk // 128
    b, qh = c // 2, c % 2
    f8 = ml_dtypes.float8_e4m3
    hb = np.roll(np.asarray(h[:, b, :], dtype=np.float32), -qh * TQ, axis=0)
    maskb = np.roll(np.asarray(attn_mask[:, b]), -qh * TQ)
    idx = np.nonzero(~maskb)[0]
    nk = idx.shape[0]
    assert nk <= tk
    idxp = np.concatenate([idx, np.zeros(tk - nk, np.int64)])
    hbT = np.ascontiguousarray(hb.T).astype(f8)             # [D, T]

    def sb(a):
        # [DC*128, N] -> SBUF layout [128, DC, N]
        dc = a.shape[0] // 128
        return np.ascontiguousarray(
            a.reshape(dc, 128, a.shape[1]).swapaxes(0, 1))

    m = {}
    m["hTq"] = sb(hbT[:, :TQ])
    m["hTk"] = sb(hbT[:, idxp])
    m["hq"] = np.ascontiguousarray(hb[:TQ])                 # [TQ, D] f32
    m["wqT"] = sb((wq.T * WS).astype(f8))
    m["wkT"] = sb((wkv[:F].T * WS).astype(f8))
    m["wvT"] = sb((wkv[F:].T * WS).astype(f8))
    m["woT"] = sb((wo.T * WS).astype(f8))
    mbias = np.full(tk, NEG_BIG, np.float32)
    mbias[:nk] = 0.0
    m["maskbias"] = np.ascontiguousarray(mbias.reshape(KC, 128).T)
    m["g_rep"] = np.ascontiguousarray(
        np.broadcast_to(np.asarray(ln_g, np.float32), (128, D)))
    m["b_rep"] = np.ascontiguousarray(
        np.broadcast_to(np.asarray(ln_b, np.float32), (128, D)))
    return m

# ======================================================================
# Host-side runner: shard, compile (cached), execute on 8 cores, gather.
# ======================================================================
_NC_CACHE = {}
LAST_RESULT = None  # BassKernelResults of the most recent kernel() call


def _get_nc(T, TQ, TK, D, NH, DH, plain_ln):
    key = (T, TQ, TK, D, NH, DH, plain_ln)
    if key not in _NC_CACHE:
        _NC_CACHE[key] = build_nc(T, TQ, TK, D, NH, DH, n_cores=8,
                                  plain_ln=plain_ln, debug=False)
    return _NC_CACHE[key]


def kernel(h, attn_mask, wq, wkv, wo, ln_g, ln_b):
    """Full-input MultiHeadAttn forward on 8 NeuronCores.

    h: [T, B, D] f32; attn_mask: [T, B] bool (True = masked key);
    wq: [F, D]; wkv: [2F, D]; wo: [D, F]; ln_g/ln_b: [D].
    Returns [T, B, D] f32 = layer_norm(h + attn(h)).
    """
    from concourse.bass_utils import run_bass_kernel_spmd
    global LAST_RESULT

    h = np.asarray(h)
    attn_mask = np.asarray(attn_mask)
    wq = np.asarray(wq, np.float32)
    wkv = np.asarray(wkv, np.float32)
    wo = np.asarray(wo, np.float32)
    ln_g = np.asarray(ln_g, np.float32)
    ln_b = np.asarray(ln_b, np.float32)

    T, B, D = h.shape
    NH = 16
    DH = wq.shape[0] // NH
    assert 2 * B == 8, "sharding assumes batch 4 over 8 cores"
    TQ = T // 2
    TK = min(choose_tk(attn_mask), T)
    plain_ln = bool(np.all(ln_g == 1.0) and np.all(ln_b == 0.0))

    nc = _get_nc(T, TQ, TK, D, NH, DH, plain_ln)
    in_maps = [host_prep_core(c, TK, h, attn_mask, wq, wkv, wo, ln_g, ln_b,
                              NH=NH, DH=DH) for c in range(8)]
    # First execution after a NEFF load runs cold (DMA rings, PE clock
    # ramp); execute once to warm the device, then measure the real run.
    run_bass_kernel_spmd(nc, in_maps, core_ids=list(range(8)))
    res = run_bass_kernel_spmd(nc, in_maps, core_ids=list(range(8)))
    LAST_RESULT = res

    out = np.empty((T, B, D), np.float32)
    for c in range(8):
        b, qh = c // 2, c % 2
        out[qh * TQ:(qh + 1) * TQ, b, :] = \
            np.asarray(res.results[c]["out"]).astype(np.float32)
    return out


# revision 30
# speedup vs baseline: 1.0030x; 1.0030x over previous
"""Multi-head attention Bass/Tile kernel for TRN2, 8-core SPMD.

Sharding: core c handles batch b = c//2, query-half qh = c%2. The host
rotates the token axis per core so query rows sit at [0:TQ] (attention is
key-permutation invariant), and gathers the unmasked keys (mask compaction)
so K/V projection + attention only touch TK <= T key tokens.

The kernel is organized into 16 "ticks" of (ic, hp) = (query i-chunk,
head pair), paced by the softmax exp stream on ACT (~145us). Each tick
emits its S matmuls + exps ONE KEY-TILE AT A TIME with jobs woven in
between: the PV/den matmuls of the previous tick, projection chunks and
out-proj/LayerNorm tiles. The fine interleave matters because the PE
queue is strict FIFO and s-tiles (psA) drain at ACT's exp pace — a
batch of S matmuls would park the PE on a psA slot with runnable work
stuck behind it. Projections draw PSUM from a separate pool (psP) for
the same reason. PSUM budget: psA 2x2 banks + psP 2 + psB 2 = 8.

Projections (Q/K/V/O) run as fp8e4m3 MatmulPerfMode.DoubleRow (two
128-contraction subtiles per matmul at 0.5 cyc/row; full 128x128 tile —
DR is incompatible with col tiling). Weights are pre-scaled by 32 on
the host so U(-1/32,1/32) lands in fp8's normal range; inverse scales
fold into the PSUM->SBUF copies. Scores/exp/PV/den stay bf16.

LayerNorm statistics ride the DVE: the residual add and an x^2 pass
carry accum_out (per-partition free-dim sums), so ACT only does the
tiny [128,1] Ln/Exp for rstd. When ln_g==1 and ln_b==0 (detected on
host), the final affine is a single tensor_scalar.

Matmul layouts (out = lhsT.T @ rhs, contraction on partitions):
  QT/KT [F, *] bf16 : lhsT=w*T [128,2,128] fp8 DR, rhs=hT* [128,2,*] fp8
  V     [TK, F] bf16: lhsT=hTk [128,2,t128] fp8 DR, rhs=wvT [128,2,F]
  S^T   [j, (h0 i512 | h1 i512)] psum (2 banks): row-tiled head pair
  exp   one ACT op per j-tile: [128, 1024], bias=maskbias per-partition
  PV+den [d0:64|den 64:128, i] psum: lhsT=V[j,64]@(0,0) + ones[j,64]@(0,64)
  O     [t, D] psum : lhsT=AVT [128,2,t128] fp8 DR, rhs=woT [128,2,D]
"""
import numpy as np
import ml_dtypes

import concourse.bass as bass
import concourse.tile as tile
from concourse import bacc, mybir

F32 = mybir.dt.float32
BF16 = mybir.dt.bfloat16
FP8 = mybir.dt.float8e4
AF = mybir.ActivationFunctionType
ALU = mybir.AluOpType
DR = mybir.MatmulPerfMode.DoubleRow

NEG_BIG = -1.0e30
WS = 32.0          # host-side fp8 weight scale (wq/wk/wv/wo * 32)
AS = 8.0           # device-side AVT fp8 scale


def _pin_act_tables():
    """Force every ACT func we use (Exp, Ln, Copy) to resolve to the
    single `natural_log_exp_and_others` table set, so the kernel does
    exactly one ACT_TABLE_LOAD instead of thrashing (~2.6us per switch).
    Preserves dict order (set ids are positional)."""
    import concourse.hw_specs as hw_specs
    if getattr(hw_specs, "_mha_tables_pinned", False):
        return
    orig = hw_specs.get_activation_tables

    def patched(module_arch):
        tabs = orig(module_arch)
        pin = "natural_log_exp_and_others"
        if pin in tabs:
            pinned_funcs = tabs[pin]
            for name, fns in tabs.items():
                if name != pin:
                    tabs[name] = fns - pinned_funcs
        return tabs

    hw_specs.get_activation_tables = patched
    import concourse.bacc as bacc_mod
    bacc_mod.get_activation_tables = patched
    hw_specs._mha_tables_pinned = True


def _chunks(total, step):
    out = []
    off = 0
    while off < total:
        out.append((off, min(step, total - off)))
        off += step
    return out


def build_nc(T, TQ, TK, D, NH, DH, n_cores=8, plain_ln=True, debug=False):
    """Build the single-core SPMD Bass program. TK = compacted key count.
    plain_ln: ln_g is all-ones and ln_b all-zeros (skip the affine)."""
    F = NH * DH
    DC = D // 128        # D contraction chunks
    DP = DC // 2         # DoubleRow dc pairs
    FC = F // 128        # feature chunks (2 heads per chunk, DH=64)
    FP = FC // 2         # DoubleRow fc pairs
    KC = TK // 128       # key tiles
    TT = TQ // 128       # query t-tiles
    ICS = min(512, TQ)   # i-chunk size
    ICN = TQ // ICS
    FS = min(512, F)
    DS = min(512, D)
    assert DH == 64 and F % 128 == 0 and D % 128 == 0
    assert TQ % 128 == 0 and TK % 128 == 0 and DC % 2 == 0 and FC % 2 == 0

    _pin_act_tables()
    nc = bacc.Bacc("TRN2", target_bir_lowering=False, debug=debug,
                   num_devices=n_cores)

    # ---- DRAM I/O ----
    hTq_d = nc.dram_tensor("hTq", [128, DC, TQ], FP8, kind="ExternalInput")
    hTk_d = nc.dram_tensor("hTk", [128, DC, TK], FP8, kind="ExternalInput")
    hq_d = nc.dram_tensor("hq", [TQ, D], F32, kind="ExternalInput")
    wqT_d = nc.dram_tensor("wqT", [128, DC, F], FP8, kind="ExternalInput")
    wkT_d = nc.dram_tensor("wkT", [128, DC, F], FP8, kind="ExternalInput")
    wvT_d = nc.dram_tensor("wvT", [128, DC, F], FP8, kind="ExternalInput")
    woT_d = nc.dram_tensor("woT", [128, FC, D], FP8, kind="ExternalInput")
    mb_d = nc.dram_tensor("maskbias", [128, KC], F32, kind="ExternalInput")
    g_d = nc.dram_tensor("g_rep", [128, D], F32, kind="ExternalInput")
    b_d = nc.dram_tensor("b_rep", [128, D], F32, kind="ExternalInput")
    out_d = nc.dram_tensor("out", [TQ, D], BF16, kind="ExternalOutput")

    with tile.TileContext(nc) as tc:
        with (
            tc.tile_pool(name="hpool", bufs=1) as hpool,
            tc.tile_pool(name="wts", bufs=4) as wts,
            tc.tile_pool(name="acts", bufs=1) as acts,
            tc.tile_pool(name="small", bufs=1) as small,
            tc.tile_pool(name="exps", bufs=2) as expp,
            tc.tile_pool(name="epi", bufs=5) as epi,
            tc.tile_pool(name="psA", bufs=2, space="PSUM") as psA,
            tc.tile_pool(name="psP", bufs=2, space="PSUM") as psP,
            tc.tile_pool(name="psB", bufs=2, space="PSUM") as psB,
        ):
            # ---- persistent SBUF tiles ----
            hTq = hpool.tile([128, DC, TQ], FP8, tag="htq")
            hTk = hpool.tile([128, DC, TK], FP8, tag="htk")
            wqT = wts.tile([128, DC, F], FP8, tag="w")
            wkT = wts.tile([128, DC, F], FP8, tag="w")
            wvT = wts.tile([128, DC, F], FP8, tag="w")
            woT = wts.tile([128, FC, D], FP8, tag="w")
            QT = acts.tile([128, FC * TQ], BF16, tag="qt")
            KT = acts.tile([128, FC * TK], BF16, tag="kt")
            V = acts.tile([128, KC, F], BF16, tag="v")
            AVT = acts.tile([128, FC, TQ], FP8, tag="avt")
            ones = small.tile([128, 64], BF16, tag="ones")
            mb = small.tile([128, KC], F32, tag="mb")
            eps_t = small.tile([128, 1], F32, tag="eps")

            nc.vector.memset(ones[:], 1.0)
            nc.vector.memset(eps_t[:], 1e-5)
            nc.sync.dma_start(mb[:], mb_d[:])
            # The DRAM tensors are pre-arranged on the host in SBUF layout
            # ([128, DC, *]) so each loads in few DMAs; queues are spread
            # so Q inputs (sync/scalar) and K inputs (gpsimd) run on
            # parallel channels. The p=0 slices of wq/wk (128 cols) go
            # first so the p0 projections — and with them the ACT exp
            # stream — start as soon as hTq/hTk land, instead of waiting
            # for the full weight tensors.
            nc.sync.dma_start(wqT[:, :, 0:128], wqT_d[:, :, 0:128])
            nc.sync.dma_start(wkT[:, :, 0:128], wkT_d[:, :, 0:128])
            nc.scalar.dma_start(hTq[:], hTq_d[:])
            nc.sync.dma_start(hTk[:], hTk_d[:])
            nc.sync.dma_start(wqT[:, :, 128:F], wqT_d[:, :, 128:F])
            nc.sync.dma_start(wkT[:, :, 128:F], wkT_d[:, :, 128:F])
            nc.scalar.dma_start(wvT[:], wvT_d[:])
            # sync-queue DMAs run serially, so woT at the tail is
            # naturally deprioritized out of the critical startup burst
            # (it isn't needed until the first oln, ~halfway in).
            nc.sync.dma_start(woT[:], woT_d[:])
            g_re = small.tile([128, D], F32, tag="g")
            b_re = small.tile([128, D], F32, tag="b")
            if not plain_ln:
                nc.sync.dma_start(g_re[:], g_d[:])
                nc.sync.dma_start(b_re[:], b_d[:])

            # ---- emission helpers (fp8 DoubleRow projections) ----
            # Projections and the out-projection draw PSUM from psP so
            # they never contend with the s-tiles in psA: the PE queue is
            # strict FIFO, so an S matmul parked on an exp-paced psA slot
            # would otherwise stall every instruction emitted behind it.
            def qchunk(p, t0, tn):
                def job():
                    ps = psP.tile([128, tn], F32, tag="P")
                    for dp in range(DP):
                        nc.tensor.matmul(
                            ps[:],
                            wqT[:, 2 * dp:2 * dp + 2, p * 128:(p + 1) * 128],
                            hTq[:, 2 * dp:2 * dp + 2, t0:t0 + tn],
                            start=(dp == 0), stop=(dp == DP - 1),
                            perf_mode=DR)
                    # fold 1/WS (fp8 weight scale) and 1/8 (attn scale)
                    nc.vector.tensor_scalar(
                        QT[:, p * TQ + t0: p * TQ + t0 + tn], ps[:],
                        1.0 / (WS * 8.0), None, op0=ALU.mult)
                return job

            def kchunk(p, t0, tn):
                def job():
                    ps = psP.tile([128, tn], F32, tag="P")
                    for dp in range(DP):
                        nc.tensor.matmul(
                            ps[:],
                            wkT[:, 2 * dp:2 * dp + 2, p * 128:(p + 1) * 128],
                            hTk[:, 2 * dp:2 * dp + 2, t0:t0 + tn],
                            start=(dp == 0), stop=(dp == DP - 1),
                            perf_mode=DR)
                    nc.vector.tensor_scalar(
                        KT[:, p * TK + t0: p * TK + t0 + tn], ps[:],
                        1.0 / WS, None, op0=ALU.mult)
                return job

            def vchunk(jc, f0, fn):
                def job():
                    ps = psP.tile([128, fn], F32, tag="P")
                    for dp in range(DP):
                        nc.tensor.matmul(
                            ps[:],
                            hTk[:, 2 * dp:2 * dp + 2, jc * 128:(jc + 1) * 128],
                            wvT[:, 2 * dp:2 * dp + 2, f0:f0 + fn],
                            start=(dp == 0), stop=(dp == DP - 1),
                            perf_mode=DR)
                    nc.vector.tensor_scalar(
                        V[:, jc, f0:f0 + fn], ps[:],
                        1.0 / WS, None, op0=ALU.mult)
                return job

            def emit_s(ic, hp, e3, jc):
                """One key-tile of scores + its exp."""
                io = ic * ICS
                s = psA.tile([128, 2 * ICS], F32, tag="A")
                nc.tensor.matmul(
                    s[:, 0:ICS],
                    KT[0:64, hp * TK + jc * 128: hp * TK + (jc + 1) * 128],
                    QT[0:64, hp * TQ + io: hp * TQ + io + ICS],
                    start=True, stop=True, tile_position=(0, 0))
                nc.tensor.matmul(
                    s[:, ICS:2 * ICS],
                    KT[64:128, hp * TK + jc * 128: hp * TK + (jc + 1) * 128],
                    QT[64:128, hp * TQ + io: hp * TQ + io + ICS],
                    start=True, stop=True, tile_position=(64, 0))
                nc.scalar.activation(e3[:, jc, :], s[:], AF.Exp,
                                     bias=mb[:, jc:jc + 1])

            def pv_jobs(ic, hp, e3):
                """Per-key-tile PV + denominator jobs, then the AVT
                normalize, for a tick whose exps are (nearly) done."""
                io = ic * ICS
                h0, h1 = 2 * hp, 2 * hp + 1
                pvP = psB.tile([128, ICS], F32, tag="pv")
                pvD = psB.tile([128, ICS], F32, tag="pv")
                jobs = []
                KP = KC // 2

                def mk_pv(jc):
                    def j():
                        st, sp = (jc == 0), (jc == KC - 1)
                        nc.tensor.matmul(
                            pvP[0:64, :],
                            V[:, jc, h0 * DH:(h0 + 1) * DH],
                            e3[:, jc, 0:ICS], start=st, stop=sp,
                            tile_position=(0, 0), skip_group_check=True)
                        nc.tensor.matmul(
                            pvP[64:128, :],
                            V[:, jc, h1 * DH:(h1 + 1) * DH],
                            e3[:, jc, ICS:2 * ICS], start=st, stop=sp,
                            tile_position=(0, 64), skip_group_check=True)
                    return j

                # Pair-sum adjacent e-tiles IN PLACE on the DVE (idle
                # capacity) so the PE-side denominator reduction needs 5
                # matmul rounds instead of 9. Each add is emitted right
                # after the two PV jobs that read the original slices, so
                # all adds are long done when the den matmuls run.
                def mk_pair(k):
                    def j():
                        nc.vector.tensor_tensor(
                            e3[:, 2 * k, :], e3[:, 2 * k, :],
                            e3[:, 2 * k + 1, :], op=ALU.add)
                    return j
                for jc in range(KC):
                    jobs.append(mk_pv(jc))
                    if jc % 2 == 1:
                        jobs.append(mk_pair(jc // 2))

                def denjob():
                    idxs = [2 * k for k in range(KP)]
                    if 2 * KP != KC:
                        idxs.append(KC - 1)
                    for k, idx in enumerate(idxs):
                        st, sp = (k == 0), (k == len(idxs) - 1)
                        nc.tensor.matmul(
                            pvD[0:64, :], ones[:, 0:64],
                            e3[:, idx, 0:ICS], start=st, stop=sp,
                            tile_position=(0, 0), skip_group_check=True)
                        nc.tensor.matmul(
                            pvD[64:128, :], ones[:, 0:64],
                            e3[:, idx, ICS:2 * ICS], start=st, stop=sp,
                            tile_position=(0, 64), skip_group_check=True)
                jobs.append(denjob)

                def norm():
                    # approx-reciprocal of den then one multiply for both
                    # heads; AVT is fp8 scaled by AS (recip gets AS/den),
                    # reading PV straight from PSUM.
                    pvDc = epi.tile([128, ICS], F32, tag="rec")
                    nc.vector.tensor_scalar(pvDc[:], pvD[:], 1.0 / AS, None,
                                            op0=ALU.mult)
                    nc.vector.reciprocal_approx_fast(pvDc[:], pvDc[:])
                    nc.vector.tensor_tensor(
                        AVT[:, hp, io:io + ICS],
                        pvP[:], pvDc[:], op=ALU.mult)
                jobs.append(norm)
                return jobs

            def oln_jobs(tt, drain=False):
                """Out-projection + residual + LayerNorm for one t-tile,
                as two jobs (one per half-D chunk). LN stats come from
                DVE accum_out; ACT only does the tiny Ln/Exp for rstd."""
                st = {}
                dchunks = _chunks(D, DS)

                def ochunk(ci, d0, dn):
                    ps = psP.tile([128, dn], F32, tag="P")
                    for fp_ in range(FP):
                        nc.tensor.matmul(
                            ps[:],
                            AVT[:, 2 * fp_:2 * fp_ + 2,
                                tt * 128:(tt + 1) * 128],
                            woT[:, 2 * fp_:2 * fp_ + 2, d0:d0 + dn],
                            start=(fp_ == 0), stop=(fp_ == FP - 1),
                            perf_mode=DR)
                    # x = attn_out + h; 1/(WS*AS) undoes wo and AVT scales
                    nc.vector.scalar_tensor_tensor(
                        st['x'][:, d0:d0 + dn], ps[:], 1.0 / (WS * AS),
                        st['hqt'][:, d0:d0 + dn], op0=ALU.mult, op1=ALU.add,
                        accum_out=st['stats'][:, ci:ci + 1])

                def a():
                    st['x'] = epi.tile([128, D], F32, tag="x", name="x")
                    st['hqt'] = epi.tile([128, D], F32, tag="hqt", name="hqt")
                    st['stats'] = epi.tile([128, 8], F32, tag="stats", name="stats")
                    nc.sync.dma_start(st['hqt'][:],
                                      hq_d[tt * 128:(tt + 1) * 128, :])
                    ochunk(0, *dchunks[0])

                def b():
                    x, hqt, stats = st['x'], st['hqt'], st['stats']
                    for ci, (d0, dn) in enumerate(dchunks[1:], start=1):
                        ochunk(ci, d0, dn)
                    # sum(x^2) (uncentered variance); hqt is dead, use it
                    # as the mandatory elementwise output scratch. In the
                    # drain, ACT is idle (exp stream over) while the DVE
                    # LN chains serialize — run the x^2 pass there.
                    if drain:
                        nc.scalar.activation(hqt[:], x[:], AF.Square,
                                             accum_out=stats[:, 2:3])
                    else:
                        nc.vector.scalar_tensor_tensor(
                            hqt[:], x[:], 1.0, x[:],
                            op0=ALU.mult, op1=ALU.mult,
                            accum_out=stats[:, 2:3])
                    mu = stats[:, 3:4]
                    nc.vector.tensor_tensor(mu, stats[:, 0:1],
                                            stats[:, 1:2], op=ALU.add)
                    nc.vector.tensor_scalar(mu, mu, 1.0 / D, None,
                                            op0=ALU.mult)
                    mu2 = stats[:, 4:5]
                    nc.vector.tensor_scalar(mu2, mu, mu, None, op0=ALU.mult)
                    msq = stats[:, 5:6]
                    nc.vector.tensor_scalar(msq, stats[:, 2:3], 1.0 / D,
                                            None, op0=ALU.mult)
                    var = stats[:, 6:7]
                    nc.vector.tensor_tensor(var, msq, mu2, op=ALU.subtract)
                    # rstd = exp(-0.5*ln(var+eps)): Ln+Exp live in one ACT
                    # table set with the attention Exps -> no table thrash
                    lnv = stats[:, 0:1]
                    nc.scalar.activation(lnv, var, AF.Ln, bias=eps_t[:])
                    rstd = stats[:, 1:2]
                    nc.scalar.activation(rstd, lnv, AF.Exp, scale=-0.5)
                    nmr = stats[:, 4:5]
                    nc.vector.tensor_scalar(nmr, mu, rstd, -1.0,
                                            op0=ALU.mult, op1=ALU.mult)
                    # xhat = x*rstd - mu*rstd into a bf16 staging tile
                    # (halves the output DMA bytes), in half-D pieces so
                    # the first out-DMA overlaps the second affine. Drain
                    # tiles compute xhat on the now-idle ACT engine
                    # (Identity with per-partition scale/bias) to shorten
                    # the serialized DVE tail.
                    xo = epi.tile([128, D], BF16, tag="xo", name="xo")
                    for ci, (d0, dn) in enumerate(dchunks):
                        if plain_ln and drain and ci == 0:
                            nc.scalar.activation(
                                xo[:, d0:d0 + dn], x[:, d0:d0 + dn],
                                AF.Identity, scale=rstd, bias=nmr)
                        elif plain_ln:
                            nc.vector.tensor_scalar(
                                xo[:, d0:d0 + dn], x[:, d0:d0 + dn], rstd,
                                nmr, op0=ALU.mult, op1=ALU.add)
                        else:
                            nc.vector.tensor_scalar(
                                x[:, d0:d0 + dn], x[:, d0:d0 + dn], rstd,
                                nmr, op0=ALU.mult, op1=ALU.add)
                            nc.vector.scalar_tensor_tensor(
                                x[:, d0:d0 + dn], x[:, d0:d0 + dn], 1.0,
                                g_re[:, d0:d0 + dn],
                                op0=ALU.mult, op1=ALU.mult)
                            nc.vector.tensor_tensor(
                                xo[:, d0:d0 + dn], x[:, d0:d0 + dn],
                                b_re[:, d0:d0 + dn], op=ALU.add)
                        nc.sync.dma_start(
                            out_d[tt * 128:(tt + 1) * 128, d0:d0 + dn],
                            xo[:, d0:d0 + dn])
                return [a, b]

            # ---- ACT-anchored tick schedule with fine interleave ----
            # warmup: QT/KT chunk 0 so tick (0,0)'s scores start early
            for t0, tn in _chunks(TQ, 512):
                qchunk(0, t0, tn)()
            for t0, tn in _chunks(TK, 512):
                kchunk(0, t0, tn)()

            ticks = [(ic, hp) for ic in range(ICN) for hp in range(FC)]
            # V-proj chunks: f-half 0 is needed by PV of hp 0..FC/2-1,
            # f-half 1 by hp FC/2.. — front-load half 0, spread half 1.
            vjobs1 = [(jc, FS, F - FS) for jc in range(KC)] if F > FS else []
            prev = None
            for t, (ic, hp) in enumerate(ticks):
                jobs = []
                if prev is not None:
                    jobs += pv_jobs(*prev)
                if ic == 0:
                    if hp == 0:
                        jobs += [vchunk(jc, 0, FS) for jc in range(KC)]
                    elif hp in (1, 2, 3) and vjobs1:
                        k = (len(vjobs1) + 2) // 3
                        jobs += [vchunk(*vj)
                                 for vj in vjobs1[(hp - 1) * k: hp * k]]
                    if hp + 1 < FC:
                        jobs += [qchunk(hp + 1, t0, tn)
                                 for t0, tn in _chunks(TQ, 512)]
                        jobs += [kchunk(hp + 1, t0, tn)
                                 for t0, tn in _chunks(TK, 512)]
                if ic >= 1 and hp % 2 == 1:
                    # olns of the previous ic: 4 t-tiles over ticks 1,3,5,7
                    jobs += oln_jobs((ic - 1) * ICS // 128 + (hp - 1) // 2)
                # Emit S one key-tile at a time with jobs woven between,
                # so the exp-paced psA slots never park the PE FIFO while
                # runnable work sits behind them.
                e3 = expp.tile([128, KC, 2 * ICS], BF16, tag="e")
                emit_s(ic, hp, e3, 0)
                if KC > 1:
                    emit_s(ic, hp, e3, 1)
                ngaps = max(KC - 2, 1)
                per = -(-len(jobs) // ngaps) if jobs else 0
                gi = 0
                for jc in range(2, KC):
                    for job in jobs[gi:gi + per]:
                        job()
                    gi += per
                    emit_s(ic, hp, e3, jc)
                for job in jobs[gi:]:
                    job()
                prev = (ic, hp, e3)
            # drain: PV of the last tick, then the final ic's olns
            for job in pv_jobs(*prev):
                job()
            drain_jobs = [oln_jobs(tt, drain=True)
                          for tt in range((ICN - 1) * ICS // 128,
                                          TQ // 128)]
            for a, _ in drain_jobs:
                a()
            for _, b in drain_jobs:
                b()

    nc.compile()
    return nc


def choose_tk(attn_mask):
    """Compacted key count: max unmasked count over batches, ceil to 128."""
    m = np.asarray(attn_mask)
    counts = (~m).sum(axis=0)
    tk = int(((int(counts.max()) + 127) // 128) * 128)
    return max(tk, 128)


def host_prep_core(c, tk, h, attn_mask, wq, wkv, wo, ln_g, ln_b, NH=16, DH=64):
    """Build the per-core input map (numpy) for core c."""
    T, B, D = h.shape
    F = NH * DH
    TQ = T // 2
    KC = tk // 128
    b, qh = c // 2, c % 2
    f8 = ml_dtypes.float8_e4m3
    hb = np.roll(np.asarray(h[:, b, :], dtype=np.float32), -qh * TQ, axis=0)
    maskb = np.roll(np.asarray(attn_mask[:, b]), -qh * TQ)
    idx = np.nonzero(~maskb)[0]
    nk = idx.shape[0]
    assert nk <= tk
    idxp = np.concatenate([idx, np.zeros(tk - nk, np.int64)])
    hbT = np.ascontiguousarray(hb.T).astype(f8)             # [D, T]

    def sb(a):
        # [DC*128, N] -> SBUF layout [128, DC, N]
        dc = a.shape[0] // 128
        return np.ascontiguousarray(
            a.reshape(dc, 128, a.shape[1]).swapaxes(0, 1))

    m = {}
    m["hTq"] = sb(hbT[:, :TQ])
    m["hTk"] = sb(hbT[:, idxp])
    m["hq"] = np.ascontiguousarray(hb[:TQ])                 # [TQ, D] f32
    m["wqT"] = sb((wq.T * WS).astype(f8))
    m["wkT"] = sb((wkv[:F].T * WS).astype(f8))
    m["wvT"] = sb((wkv[F:].T * WS).astype(f8))
    m["woT"] = sb((wo.T * WS).astype(f8))
    mbias = np.full(tk, NEG_BIG, np.float32)
    mbias[:nk] = 0.0
    m["maskbias"] = np.ascontiguousarray(mbias.reshape(KC, 128).T)
    m["g_rep"] = np.ascontiguousarray(
        np.broadcast_to(np.asarray(ln_g, np.float32), (128, D)))
    m["b_rep"] = np.ascontiguousarray(
        np.broadcast_to(np.asarray(ln_b, np.float32), (128, D)))
    return m

# ======================================================================
# Host-side runner: shard, compile (cached), execute on 8 cores, gather.
# ======================================================================
_NC_CACHE = {}
LAST_RESULT = None  # BassKernelResults of the most recent kernel() call


def _get_nc(T, TQ, TK, D, NH, DH, plain_ln):
    key = (T, TQ, TK, D, NH, DH, plain_ln)
    if key not in _NC_CACHE:
        _NC_CACHE[key] = build_nc(T, TQ, TK, D, NH, DH, n_cores=8,
                                  plain_ln=plain_ln, debug=False)
    return _NC_CACHE[key]


def kernel(h, attn_mask, wq, wkv, wo, ln_g, ln_b):
    """Full-input MultiHeadAttn forward on 8 NeuronCores.

    h: [T, B, D] f32; attn_mask: [T, B] bool (True = masked key);
    wq: [F, D]; wkv: [2F, D]; wo: [D, F]; ln_g/ln_b: [D].
    Returns [T, B, D] f32 = layer_norm(h + attn(h)).
    """
    from concourse.bass_utils import run_bass_kernel_spmd
    global LAST_RESULT

    h = np.asarray(h)
    attn_mask = np.asarray(attn_mask)
    wq = np.asarray(wq, np.float32)
    wkv = np.asarray(wkv, np.float32)
    wo = np.asarray(wo, np.float32)
    ln_g = np.asarray(ln_g, np.float32)
    ln_b = np.asarray(ln_b, np.float32)

    T, B, D = h.shape
    NH = 16
    DH = wq.shape[0] // NH
    assert 2 * B == 8, "sharding assumes batch 4 over 8 cores"
    TQ = T // 2
    TK = min(choose_tk(attn_mask), T)
    plain_ln = bool(np.all(ln_g == 1.0) and np.all(ln_b == 0.0))

    nc = _get_nc(T, TQ, TK, D, NH, DH, plain_ln)
    in_maps = [host_prep_core(c, TK, h, attn_mask, wq, wkv, wo, ln_g, ln_b,
                              NH=NH, DH=DH) for c in range(8)]
    # First execution after a NEFF load runs cold (DMA rings, PE clock
    # ramp); execute once to warm the device, then measure the real run.
    run_bass_kernel_spmd(nc, in_maps, core_ids=list(range(8)))
    res = run_bass_kernel_spmd(nc, in_maps, core_ids=list(range(8)))
    LAST_RESULT = res

    out = np.empty((T, B, D), np.float32)
    for c in range(8):
        b, qh = c // 2, c % 2
        out[qh * TQ:(qh + 1) * TQ, b, :] = \
            np.asarray(res.results[c]["out"]).astype(np.float32)
    return out


# revision 31
# speedup vs baseline: 1.0127x; 1.0097x over previous
"""Multi-head attention Bass/Tile kernel for TRN2, 8-core SPMD.

Sharding: core c handles batch b = c//2, query-half qh = c%2. The host
rotates the token axis per core so query rows sit at [0:TQ] (attention is
key-permutation invariant), and gathers the unmasked keys (mask compaction)
so K/V projection + attention only touch TK <= T key tokens.

The kernel is organized into 16 "ticks" of (ic, hp) = (query i-chunk,
head pair), paced by the softmax exp stream on ACT (~145us). Each tick
emits its S matmuls + exps ONE KEY-TILE AT A TIME with jobs woven in
between: the PV/den matmuls of the previous tick, projection chunks and
out-proj/LayerNorm tiles. The fine interleave matters because the PE
queue is strict FIFO and s-tiles (psA) drain at ACT's exp pace — a
batch of S matmuls would park the PE on a psA slot with runnable work
stuck behind it. Projections draw PSUM from a separate pool (psP) for
the same reason. PSUM budget: psA 2x2 banks + psP 2 + psB 2 = 8.

Projections (Q/K/V/O) run as fp8e4m3 MatmulPerfMode.DoubleRow (two
128-contraction subtiles per matmul at 0.5 cyc/row; full 128x128 tile —
DR is incompatible with col tiling). Weights are pre-scaled by 32 on
the host so U(-1/32,1/32) lands in fp8's normal range; inverse scales
fold into the PSUM->SBUF copies. Scores/exp/PV/den stay bf16.

LayerNorm statistics ride the DVE: the residual add and an x^2 pass
carry accum_out (per-partition free-dim sums), so ACT only does the
tiny [128,1] Ln/Exp for rstd. When ln_g==1 and ln_b==0 (detected on
host), the final affine is a single tensor_scalar.

Matmul layouts (out = lhsT.T @ rhs, contraction on partitions):
  QT/KT [F, *] bf16 : lhsT=w*T [128,2,128] fp8 DR, rhs=hT* [128,2,*] fp8
  V     [TK, F] bf16: lhsT=hTk [128,2,t128] fp8 DR, rhs=wvT [128,2,F]
  S^T   [j, (h0 i512 | h1 i512)] psum (2 banks): row-tiled head pair
  exp   one ACT op per j-tile: [128, 1024], bias=maskbias per-partition
  PV+den [d0:64|den 64:128, i] psum: lhsT=V[j,64]@(0,0) + ones[j,64]@(0,64)
  O     [t, D] psum : lhsT=AVT [128,2,t128] fp8 DR, rhs=woT [128,2,D]
"""
import numpy as np
import ml_dtypes

import concourse.bass as bass
import concourse.tile as tile
from concourse import bacc, mybir

F32 = mybir.dt.float32
BF16 = mybir.dt.bfloat16
FP8 = mybir.dt.float8e4
AF = mybir.ActivationFunctionType
ALU = mybir.AluOpType
DR = mybir.MatmulPerfMode.DoubleRow

NEG_BIG = -1.0e30
WS = 32.0          # host-side fp8 weight scale (wq/wk/wv/wo * 32)
AS = 8.0           # device-side AVT fp8 scale


def _pin_act_tables():
    """Force every ACT func we use (Exp, Ln, Copy) to resolve to the
    single `natural_log_exp_and_others` table set, so the kernel does
    exactly one ACT_TABLE_LOAD instead of thrashing (~2.6us per switch).
    Preserves dict order (set ids are positional)."""
    import concourse.hw_specs as hw_specs
    if getattr(hw_specs, "_mha_tables_pinned", False):
        return
    orig = hw_specs.get_activation_tables

    def patched(module_arch):
        tabs = orig(module_arch)
        pin = "natural_log_exp_and_others"
        if pin in tabs:
            pinned_funcs = tabs[pin]
            for name, fns in tabs.items():
                if name != pin:
                    tabs[name] = fns - pinned_funcs
        return tabs

    hw_specs.get_activation_tables = patched
    import concourse.bacc as bacc_mod
    bacc_mod.get_activation_tables = patched
    hw_specs._mha_tables_pinned = True


def _chunks(total, step):
    out = []
    off = 0
    while off < total:
        out.append((off, min(step, total - off)))
        off += step
    return out


def build_nc(T, TQ, TK, D, NH, DH, n_cores=8, plain_ln=True, debug=False):
    """Build the single-core SPMD Bass program. TK = compacted key count.
    plain_ln: ln_g is all-ones and ln_b all-zeros (skip the affine)."""
    F = NH * DH
    DC = D // 128        # D contraction chunks
    DP = DC // 2         # DoubleRow dc pairs
    FC = F // 128        # feature chunks (2 heads per chunk, DH=64)
    FP = FC // 2         # DoubleRow fc pairs
    KC = TK // 128       # key tiles
    TT = TQ // 128       # query t-tiles
    ICS = min(512, TQ)   # i-chunk size
    ICN = TQ // ICS
    FS = min(512, F)
    DS = min(512, D)
    assert DH == 64 and F % 128 == 0 and D % 128 == 0
    assert TQ % 128 == 0 and TK % 128 == 0 and DC % 2 == 0 and FC % 2 == 0

    _pin_act_tables()
    nc = bacc.Bacc("TRN2", target_bir_lowering=False, debug=debug,
                   num_devices=n_cores)

    # ---- DRAM I/O ----
    hTq_d = nc.dram_tensor("hTq", [128, DC, TQ], FP8, kind="ExternalInput")
    hTk_d = nc.dram_tensor("hTk", [128, DC, TK], FP8, kind="ExternalInput")
    hq_d = nc.dram_tensor("hq", [TQ, D], F32, kind="ExternalInput")
    wqT_d = nc.dram_tensor("wqT", [128, DC, F], FP8, kind="ExternalInput")
    wkT_d = nc.dram_tensor("wkT", [128, DC, F], FP8, kind="ExternalInput")
    wvT_d = nc.dram_tensor("wvT", [128, DC, F], FP8, kind="ExternalInput")
    woT_d = nc.dram_tensor("woT", [128, FC, D], FP8, kind="ExternalInput")
    mb_d = nc.dram_tensor("maskbias", [128, KC], F32, kind="ExternalInput")
    g_d = nc.dram_tensor("g_rep", [128, D], F32, kind="ExternalInput")
    b_d = nc.dram_tensor("b_rep", [128, D], F32, kind="ExternalInput")
    out_d = nc.dram_tensor("out", [TQ, D], BF16, kind="ExternalOutput")

    with tile.TileContext(nc) as tc:
        with (
            tc.tile_pool(name="hpool", bufs=1) as hpool,
            tc.tile_pool(name="wts", bufs=4) as wts,
            tc.tile_pool(name="acts", bufs=1) as acts,
            tc.tile_pool(name="small", bufs=1) as small,
            tc.tile_pool(name="exps", bufs=2) as expp,
            tc.tile_pool(name="epi", bufs=5) as epi,
            tc.tile_pool(name="psA", bufs=2, space="PSUM") as psA,
            tc.tile_pool(name="psP", bufs=2, space="PSUM") as psP,
            tc.tile_pool(name="psB", bufs=2, space="PSUM") as psB,
        ):
            # ---- persistent SBUF tiles ----
            hTq = hpool.tile([128, DC, TQ], FP8, tag="htq")
            hTk = hpool.tile([128, DC, TK], FP8, tag="htk")
            wqT = wts.tile([128, DC, F], FP8, tag="w")
            wkT = wts.tile([128, DC, F], FP8, tag="w")
            wvT = wts.tile([128, DC, F], FP8, tag="w")
            woT = wts.tile([128, FC, D], FP8, tag="w")
            QT = acts.tile([128, FC * TQ], BF16, tag="qt")
            KT = acts.tile([128, FC * TK], BF16, tag="kt")
            V = acts.tile([128, KC, F], BF16, tag="v")
            AVT = acts.tile([128, FC, TQ], FP8, tag="avt")
            ones = small.tile([128, 64], BF16, tag="ones")
            mb = small.tile([128, KC], F32, tag="mb")
            eps_t = small.tile([128, 1], F32, tag="eps")

            nc.vector.memset(ones[:], 1.0)
            nc.vector.memset(eps_t[:], 1e-5)
            nc.sync.dma_start(mb[:], mb_d[:])
            # The DRAM tensors are pre-arranged on the host in SBUF layout
            # ([128, DC, *]) so each loads in few DMAs; queues are spread
            # so Q inputs (sync/scalar) and K inputs (gpsimd) run on
            # parallel channels. The p=0 slices of wq/wk (128 cols) go
            # first so the p0 projections — and with them the ACT exp
            # stream — start as soon as hTq/hTk land, instead of waiting
            # for the full weight tensors.
            nc.sync.dma_start(wqT[:, :, 0:128], wqT_d[:, :, 0:128])
            nc.sync.dma_start(wkT[:, :, 0:128], wkT_d[:, :, 0:128])
            nc.scalar.dma_start(hTq[:], hTq_d[:])
            nc.sync.dma_start(hTk[:], hTk_d[:])
            nc.sync.dma_start(wqT[:, :, 128:F], wqT_d[:, :, 128:F])
            nc.sync.dma_start(wkT[:, :, 128:F], wkT_d[:, :, 128:F])
            nc.scalar.dma_start(wvT[:], wvT_d[:])
            # sync-queue DMAs run serially, so woT at the tail is
            # naturally deprioritized out of the critical startup burst
            # (it isn't needed until the first oln, ~halfway in).
            nc.sync.dma_start(woT[:], woT_d[:])
            g_re = small.tile([128, D], F32, tag="g")
            b_re = small.tile([128, D], F32, tag="b")
            if not plain_ln:
                nc.sync.dma_start(g_re[:], g_d[:])
                nc.sync.dma_start(b_re[:], b_d[:])

            # ---- emission helpers (fp8 DoubleRow projections) ----
            # Projections and the out-projection draw PSUM from psP so
            # they never contend with the s-tiles in psA: the PE queue is
            # strict FIFO, so an S matmul parked on an exp-paced psA slot
            # would otherwise stall every instruction emitted behind it.
            def qchunk(p, t0, tn):
                def job():
                    ps = psP.tile([128, tn], F32, tag="P")
                    for dp in range(DP):
                        nc.tensor.matmul(
                            ps[:],
                            wqT[:, 2 * dp:2 * dp + 2, p * 128:(p + 1) * 128],
                            hTq[:, 2 * dp:2 * dp + 2, t0:t0 + tn],
                            start=(dp == 0), stop=(dp == DP - 1),
                            perf_mode=DR)
                    # fold 1/WS (fp8 weight scale) and 1/8 (attn scale)
                    nc.vector.tensor_scalar(
                        QT[:, p * TQ + t0: p * TQ + t0 + tn], ps[:],
                        1.0 / (WS * 8.0), None, op0=ALU.mult)
                return job

            def kchunk(p, t0, tn):
                def job():
                    ps = psP.tile([128, tn], F32, tag="P")
                    for dp in range(DP):
                        nc.tensor.matmul(
                            ps[:],
                            wkT[:, 2 * dp:2 * dp + 2, p * 128:(p + 1) * 128],
                            hTk[:, 2 * dp:2 * dp + 2, t0:t0 + tn],
                            start=(dp == 0), stop=(dp == DP - 1),
                            perf_mode=DR)
                    nc.vector.tensor_scalar(
                        KT[:, p * TK + t0: p * TK + t0 + tn], ps[:],
                        1.0 / WS, None, op0=ALU.mult)
                return job

            def vchunk(jc, f0, fn):
                def job():
                    ps = psP.tile([128, fn], F32, tag="P")
                    for dp in range(DP):
                        nc.tensor.matmul(
                            ps[:],
                            hTk[:, 2 * dp:2 * dp + 2, jc * 128:(jc + 1) * 128],
                            wvT[:, 2 * dp:2 * dp + 2, f0:f0 + fn],
                            start=(dp == 0), stop=(dp == DP - 1),
                            perf_mode=DR)
                    nc.vector.tensor_scalar(
                        V[:, jc, f0:f0 + fn], ps[:],
                        1.0 / WS, None, op0=ALU.mult)
                return job

            def emit_s(ic, hp, e3, jc):
                """One key-tile of scores + its exp."""
                io = ic * ICS
                s = psA.tile([128, 2 * ICS], F32, tag="A")
                nc.tensor.matmul(
                    s[:, 0:ICS],
                    KT[0:64, hp * TK + jc * 128: hp * TK + (jc + 1) * 128],
                    QT[0:64, hp * TQ + io: hp * TQ + io + ICS],
                    start=True, stop=True, tile_position=(0, 0))
                nc.tensor.matmul(
                    s[:, ICS:2 * ICS],
                    KT[64:128, hp * TK + jc * 128: hp * TK + (jc + 1) * 128],
                    QT[64:128, hp * TQ + io: hp * TQ + io + ICS],
                    start=True, stop=True, tile_position=(64, 0))
                nc.scalar.activation(e3[:, jc, :], s[:], AF.Exp,
                                     bias=mb[:, jc:jc + 1])

            def pv_jobs(ic, hp, e3):
                """Per-key-tile PV + denominator jobs, then the AVT
                normalize, for a tick whose exps are (nearly) done."""
                io = ic * ICS
                h0, h1 = 2 * hp, 2 * hp + 1
                pvP = psB.tile([128, ICS], F32, tag="pv")
                pvD = psB.tile([128, ICS], F32, tag="pv")
                jobs = []
                for jc in range(KC):
                    def j(jc=jc):
                        st, sp = (jc == 0), (jc == KC - 1)
                        nc.tensor.matmul(
                            pvP[0:64, :],
                            V[:, jc, h0 * DH:(h0 + 1) * DH],
                            e3[:, jc, 0:ICS], start=st, stop=sp,
                            tile_position=(0, 0), skip_group_check=True)
                        nc.tensor.matmul(
                            pvP[64:128, :],
                            V[:, jc, h1 * DH:(h1 + 1) * DH],
                            e3[:, jc, ICS:2 * ICS], start=st, stop=sp,
                            tile_position=(0, 64), skip_group_check=True)
                        nc.tensor.matmul(
                            pvD[0:64, :], ones[:, 0:64],
                            e3[:, jc, 0:ICS], start=st, stop=sp,
                            tile_position=(0, 0), skip_group_check=True)
                        nc.tensor.matmul(
                            pvD[64:128, :], ones[:, 0:64],
                            e3[:, jc, ICS:2 * ICS], start=st, stop=sp,
                            tile_position=(0, 64), skip_group_check=True)
                    jobs.append(j)

                def norm():
                    # approx-reciprocal of den then one multiply for both
                    # heads; AVT is fp8 scaled by AS (recip gets AS/den),
                    # reading PV straight from PSUM.
                    pvDc = epi.tile([128, ICS], F32, tag="rec")
                    nc.vector.tensor_scalar(pvDc[:], pvD[:], 1.0 / AS, None,
                                            op0=ALU.mult)
                    nc.vector.reciprocal_approx_fast(pvDc[:], pvDc[:])
                    nc.vector.tensor_tensor(
                        AVT[:, hp, io:io + ICS],
                        pvP[:], pvDc[:], op=ALU.mult)
                jobs.append(norm)
                return jobs

            def oln_jobs(tt, drain=False):
                """Out-projection + residual + LayerNorm for one t-tile,
                as two jobs (one per half-D chunk). LN stats come from
                DVE accum_out; ACT only does the tiny Ln/Exp for rstd."""
                st = {}
                dchunks = _chunks(D, DS)

                def ochunk(ci, d0, dn):
                    ps = psP.tile([128, dn], F32, tag="P")
                    for fp_ in range(FP):
                        nc.tensor.matmul(
                            ps[:],
                            AVT[:, 2 * fp_:2 * fp_ + 2,
                                tt * 128:(tt + 1) * 128],
                            woT[:, 2 * fp_:2 * fp_ + 2, d0:d0 + dn],
                            start=(fp_ == 0), stop=(fp_ == FP - 1),
                            perf_mode=DR)
                    # x = attn_out + h; 1/(WS*AS) undoes wo and AVT scales
                    nc.vector.scalar_tensor_tensor(
                        st['x'][:, d0:d0 + dn], ps[:], 1.0 / (WS * AS),
                        st['hqt'][:, d0:d0 + dn], op0=ALU.mult, op1=ALU.add,
                        accum_out=st['stats'][:, ci:ci + 1])

                def a():
                    st['x'] = epi.tile([128, D], F32, tag="x", name="x")
                    st['hqt'] = epi.tile([128, D], F32, tag="hqt", name="hqt")
                    st['stats'] = epi.tile([128, 8], F32, tag="stats", name="stats")
                    nc.sync.dma_start(st['hqt'][:],
                                      hq_d[tt * 128:(tt + 1) * 128, :])
                    ochunk(0, *dchunks[0])

                def b():
                    x, hqt, stats = st['x'], st['hqt'], st['stats']
                    for ci, (d0, dn) in enumerate(dchunks[1:], start=1):
                        ochunk(ci, d0, dn)
                    # sum(x^2) (uncentered variance); hqt is dead, use it
                    # as the mandatory elementwise output scratch. In the
                    # drain, ACT is idle (exp stream over) while the DVE
                    # LN chains serialize — run the x^2 pass there.
                    if drain:
                        nc.scalar.activation(hqt[:], x[:], AF.Square,
                                             accum_out=stats[:, 2:3])
                    else:
                        nc.vector.scalar_tensor_tensor(
                            hqt[:], x[:], 1.0, x[:],
                            op0=ALU.mult, op1=ALU.mult,
                            accum_out=stats[:, 2:3])
                    mu = stats[:, 3:4]
                    nc.vector.tensor_tensor(mu, stats[:, 0:1],
                                            stats[:, 1:2], op=ALU.add)
                    nc.vector.tensor_scalar(mu, mu, 1.0 / D, None,
                                            op0=ALU.mult)
                    mu2 = stats[:, 4:5]
                    nc.vector.tensor_scalar(mu2, mu, mu, None, op0=ALU.mult)
                    msq = stats[:, 5:6]
                    nc.vector.tensor_scalar(msq, stats[:, 2:3], 1.0 / D,
                                            None, op0=ALU.mult)
                    var = stats[:, 6:7]
                    nc.vector.tensor_tensor(var, msq, mu2, op=ALU.subtract)
                    # rstd = exp(-0.5*ln(var+eps)): Ln+Exp live in one ACT
                    # table set with the attention Exps -> no table thrash
                    lnv = stats[:, 0:1]
                    nc.scalar.activation(lnv, var, AF.Ln, bias=eps_t[:])
                    rstd = stats[:, 1:2]
                    nc.scalar.activation(rstd, lnv, AF.Exp, scale=-0.5)
                    nmr = stats[:, 4:5]
                    nc.vector.tensor_scalar(nmr, mu, rstd, -1.0,
                                            op0=ALU.mult, op1=ALU.mult)
                    # xhat = x*rstd - mu*rstd into a bf16 staging tile
                    # (halves the output DMA bytes), in half-D pieces so
                    # the first out-DMA overlaps the second affine. Drain
                    # tiles compute xhat on the now-idle ACT engine
                    # (Identity with per-partition scale/bias) to shorten
                    # the serialized DVE tail.
                    xo = epi.tile([128, D], BF16, tag="xo", name="xo")
                    for ci, (d0, dn) in enumerate(dchunks):
                        if plain_ln and drain and ci == 0:
                            nc.scalar.activation(
                                xo[:, d0:d0 + dn], x[:, d0:d0 + dn],
                                AF.Identity, scale=rstd, bias=nmr)
                        elif plain_ln:
                            nc.vector.tensor_scalar(
                                xo[:, d0:d0 + dn], x[:, d0:d0 + dn], rstd,
                                nmr, op0=ALU.mult, op1=ALU.add)
                        else:
                            nc.vector.tensor_scalar(
                                x[:, d0:d0 + dn], x[:, d0:d0 + dn], rstd,
                                nmr, op0=ALU.mult, op1=ALU.add)
                            nc.vector.scalar_tensor_tensor(
                                x[:, d0:d0 + dn], x[:, d0:d0 + dn], 1.0,
                                g_re[:, d0:d0 + dn],
                                op0=ALU.mult, op1=ALU.mult)
                            nc.vector.tensor_tensor(
                                xo[:, d0:d0 + dn], x[:, d0:d0 + dn],
                                b_re[:, d0:d0 + dn], op=ALU.add)
                        nc.sync.dma_start(
                            out_d[tt * 128:(tt + 1) * 128, d0:d0 + dn],
                            xo[:, d0:d0 + dn])
                return [a, b]

            # ---- ACT-anchored tick schedule with fine interleave ----
            # warmup: QT/KT chunk 0 so tick (0,0)'s scores start early
            for t0, tn in _chunks(TQ, 512):
                qchunk(0, t0, tn)()
            for t0, tn in _chunks(TK, 512):
                kchunk(0, t0, tn)()

            ticks = [(ic, hp) for ic in range(ICN) for hp in range(FC)]
            # V-proj chunks: f-half 0 is needed by PV of hp 0..FC/2-1,
            # f-half 1 by hp FC/2.. — front-load half 0, spread half 1.
            vjobs1 = [(jc, FS, F - FS) for jc in range(KC)] if F > FS else []
            prev = None
            for t, (ic, hp) in enumerate(ticks):
                jobs = []
                if prev is not None:
                    jobs += pv_jobs(*prev)
                if ic == 0:
                    if hp == 0:
                        jobs += [vchunk(jc, 0, FS) for jc in range(KC)]
                    elif hp in (1, 2, 3) and vjobs1:
                        k = (len(vjobs1) + 2) // 3
                        jobs += [vchunk(*vj)
                                 for vj in vjobs1[(hp - 1) * k: hp * k]]
                    if hp + 1 < FC:
                        jobs += [qchunk(hp + 1, t0, tn)
                                 for t0, tn in _chunks(TQ, 512)]
                        jobs += [kchunk(hp + 1, t0, tn)
                                 for t0, tn in _chunks(TK, 512)]
                if ic >= 1 and hp % 2 == 1:
                    # olns of the previous ic: 4 t-tiles over ticks 1,3,5,7
                    jobs += oln_jobs((ic - 1) * ICS // 128 + (hp - 1) // 2)
                # Emit S one key-tile at a time with jobs woven between,
                # so the exp-paced psA slots never park the PE FIFO while
                # runnable work sits behind them.
                e3 = expp.tile([128, KC, 2 * ICS], BF16, tag="e")
                emit_s(ic, hp, e3, 0)
                if KC > 1:
                    emit_s(ic, hp, e3, 1)
                ngaps = max(KC - 2, 1)
                per = -(-len(jobs) // ngaps) if jobs else 0
                gi = 0
                for jc in range(2, KC):
                    for job in jobs[gi:gi + per]:
                        job()
                    gi += per
                    emit_s(ic, hp, e3, jc)
                for job in jobs[gi:]:
                    job()
                prev = (ic, hp, e3)
            # drain: PV of the last tick, then the final ic's olns
            for job in pv_jobs(*prev):
                job()
            drain_jobs = [oln_jobs(tt, drain=True)
                          for tt in range((ICN - 1) * ICS // 128,
                                          TQ // 128)]
            for a, _ in drain_jobs:
                a()
            for _, b in drain_jobs:
                b()

    nc.compile()
    return nc


def choose_tk(attn_mask):
    """Compacted key count: max unmasked count over batches, ceil to 128."""
    m = np.asarray(attn_mask)
    counts = (~m).sum(axis=0)
    tk = int(((int(counts.max()) + 127) // 128) * 128)
    return max(tk, 128)


def host_prep_core(c, tk, h, attn_mask, wq, wkv, wo, ln_g, ln_b, NH=16, DH=64):
    """Build the per-core input map (numpy) for core c."""
    T, B, D = h.shape
    F = NH * DH
    TQ = T // 2
    KC = tk // 128
    b, qh = c // 2, c % 2
    f8 = ml_dtypes.float8_e4m3
    hb = np.roll(np.asarray(h[:, b, :], dtype=np.float32), -qh * TQ, axis=0)
    maskb = np.roll(np.asarray(attn_mask[:, b]), -qh * TQ)
    idx = np.nonzero(~maskb)[0]
    nk = idx.shape[0]
    assert nk <= tk
    idxp = np.concatenate([idx, np.zeros(tk - nk, np.int64)])
    hbT = np.ascontiguousarray(hb.T).astype(f8)             # [D, T]

    def sb(a):
        # [DC*128, N] -> SBUF layout [128, DC, N]
        dc = a.shape[0] // 128
        return np.ascontiguousarray(
            a.reshape(dc, 128, a.shape[1]).swapaxes(0, 1))

    m = {}
    m["hTq"] = sb(hbT[:, :TQ])
    m["hTk"] = sb(hbT[:, idxp])
    m["hq"] = np.ascontiguousarray(hb[:TQ])                 # [TQ, D] f32
    m["wqT"] = sb((wq.T * WS).astype(f8))
    m["wkT"] = sb((wkv[:F].T * WS).astype(f8))
    m["wvT"] = sb((wkv[F:].T * WS).astype(f8))
    m["woT"] = sb((wo.T * WS).astype(f8))
    mbias = np.full(tk, NEG_BIG, np.float32)
    mbias[:nk] = 0.0
    m["maskbias"] = np.ascontiguousarray(mbias.reshape(KC, 128).T)
    m["g_rep"] = np.ascontiguousarray(
        np.broadcast_to(np.asarray(ln_g, np.float32), (128, D)))
    m["b_rep"] = np.ascontiguousarray(
        np.broadcast_to(np.asarray(ln_b, np.float32), (128, D)))
    return m

# ======================================================================
# Host-side runner: shard, compile (cached), execute on 8 cores, gather.
# ======================================================================
_NC_CACHE = {}
LAST_RESULT = None  # BassKernelResults of the most recent kernel() call


def _get_nc(T, TQ, TK, D, NH, DH, plain_ln):
    key = (T, TQ, TK, D, NH, DH, plain_ln)
    if key not in _NC_CACHE:
        _NC_CACHE[key] = build_nc(T, TQ, TK, D, NH, DH, n_cores=8,
                                  plain_ln=plain_ln, debug=False)
    return _NC_CACHE[key]


def kernel(h, attn_mask, wq, wkv, wo, ln_g, ln_b):
    """Full-input MultiHeadAttn forward on 8 NeuronCores.

    h: [T, B, D] f32; attn_mask: [T, B] bool (True = masked key);
    wq: [F, D]; wkv: [2F, D]; wo: [D, F]; ln_g/ln_b: [D].
    Returns [T, B, D] f32 = layer_norm(h + attn(h)).
    """
    from concourse.bass_utils import run_bass_kernel_spmd
    global LAST_RESULT

    h = np.asarray(h)
    attn_mask = np.asarray(attn_mask)
    wq = np.asarray(wq, np.float32)
    wkv = np.asarray(wkv, np.float32)
    wo = np.asarray(wo, np.float32)
    ln_g = np.asarray(ln_g, np.float32)
    ln_b = np.asarray(ln_b, np.float32)

    T, B, D = h.shape
    NH = 16
    DH = wq.shape[0] // NH
    assert 2 * B == 8, "sharding assumes batch 4 over 8 cores"
    TQ = T // 2
    TK = min(choose_tk(attn_mask), T)
    plain_ln = bool(np.all(ln_g == 1.0) and np.all(ln_b == 0.0))

    nc = _get_nc(T, TQ, TK, D, NH, DH, plain_ln)
    in_maps = [host_prep_core(c, TK, h, attn_mask, wq, wkv, wo, ln_g, ln_b,
                              NH=NH, DH=DH) for c in range(8)]
    # First execution after a NEFF load runs cold (DMA rings, PE clock
    # ramp); execute once to warm the device, then measure the real run.
    run_bass_kernel_spmd(nc, in_maps, core_ids=list(range(8)))
    res = run_bass_kernel_spmd(nc, in_maps, core_ids=list(range(8)))
    LAST_RESULT = res

    out = np.empty((T, B, D), np.float32)
    for c in range(8):
        b, qh = c // 2, c % 2
        out[qh * TQ:(qh + 1) * TQ, b, :] = \
            np.asarray(res.results[c]["out"]).astype(np.float32)
    return out
